# revision 1
# baseline (speedup 1.0000x reference)
"""DiT block kernel for 8 Trainium2 NeuronCores (self-contained).

Sharding: sequence-parallel over padded S (3600 -> 4096, 512 rows/core) for
everything: LN/modulate/qkvo/attention/cross-attn AND the FFN (each core runs
its 512 rows through the full 8960-dim hidden layer, streaming w1/w2).
Collectives: AllGather(kT), AllGather(v-rows) for self-attention only.
Attention/projections in bf16; FFN matmuls in fp8-e4m3 DoubleRow (2x PE).
fp32 accumulate everywhere, residual spine fp32.

Key scheduling choices (see memory notes): v projected directly in row
orientation (no PE transposes), per-head-contiguous AllGather layouts so
attention k/v loads are two big 1KB-line DMAs per head, exp over paired
score tiles (one Act op per two kpos tiles), softmax denominators as bf16
DVE accumulators, weight streams batched + multi-buffered (DMA dispatch
costs ~625ns of shared HWDGE per instruction), export DMAs issued from the
Act queue (in-order queues: a waiting DMA blocks its whole queue).
"""

import numpy as np
import ml_dtypes

import concourse.bacc as bacc
import concourse.bass as bass
import concourse.mybir as mybir
import concourse.tile as tile
from concourse.masks import make_identity
from concourse.bass_utils import run_bass_kernel_spmd

F32 = mybir.dt.float32
BF16 = mybir.dt.bfloat16
FP8 = mybir.dt.float8e4
AF = mybir.ActivationFunctionType
ALU = mybir.AluOpType
DROW = mybir.MatmulPerfMode.DoubleRow

N_CORES = 8
S = 3600
SP = 4096            # padded sequence
SH = 512             # rows per core
D = 1536
H = 12
HD = 128
LC = 512             # context length
FFN = 8960
FM = FFN // 128      # 70 hidden tiles
NKT = 29             # kpos tiles covering rows 0..3712 (>=3600)
EPS = 1e-6
SCALE = float(HD) ** -0.5
NEG = -80.0          # additive mask for padded k positions

BF = ml_dtypes.bfloat16
F8 = ml_dtypes.float8_e4m3
FFN_FP8 = True       # fp8-e4m3 DoubleRow FFN matmuls
SIM_MODE = False     # replace collectives with local DMAs (for TimelineSim)


def build():
    nc = bacc.Bacc(num_devices=N_CORES)

    # ---------------- I/O ----------------
    io = {}
    io["x_sh"] = nc.dram_tensor("x_sh", [SH, D], F32, kind="ExternalInput")
    io["ctx_bf"] = nc.dram_tensor("ctx_bf", [LC, D], BF16, kind="ExternalInput")
    io["modul"] = nc.dram_tensor("modul", [6, D], F32, kind="ExternalInput")
    io["t_mod"] = nc.dram_tensor("t_mod", [6, D], F32, kind="ExternalInput")
    io["cos_dup"] = nc.dram_tensor("cos_dup", [128, SH], BF16, kind="ExternalInput")
    io["sin_dup"] = nc.dram_tensor("sin_dup", [128, SH], BF16, kind="ExternalInput")
    io["kmask"] = nc.dram_tensor("kmask", [128, 1], F32, kind="ExternalInput")

    wname = dict(
        sa_qw_p=[H, 128, D], sa_kw_p=[H, 128, D], sa_vw_r=[H, 128, D],
        sa_ow=[H, 128, D],
        ca_qw=[H, 128, D], ca_kw=[H, 128, D], ca_vw_r=[H, 128, D],
        ca_ow=[H, 128, D],
    )
    W = {k: nc.dram_tensor(k, v, BF16, kind="ExternalInput") for k, v in wname.items()}
    fdt = FP8 if FFN_FP8 else BF16
    W["w1_s"] = nc.dram_tensor("w1_s", [128, FM, H, 128], fdt, kind="ExternalInput")
    W["w2_s"] = nc.dram_tensor("w2_s", [128, FM, D], fdt, kind="ExternalInput")

    cname = [
        "sa_qb_c", "sa_kb_c", "sa_nq_c", "sa_nk_c",
        "ca_qb_c", "ca_kb_c", "ca_nq_c", "ca_nk_c",
    ]
    C = {k: nc.dram_tensor(k, [128, H], F32, kind="ExternalInput") for k in cname}
    C["b1_c"] = nc.dram_tensor("b1_c", [128, FM], F32, kind="ExternalInput")

    rname = ["n3w_r", "n3b_r"]
    Rr = {k: nc.dram_tensor(k, [1, D], F32, kind="ExternalInput") for k in rname}
    bname = ["b2_b16", "sa_ob_b16", "ca_ob_b16", "sa_vb_b16", "ca_vb_b16"]
    for k in bname:
        Rr[k] = nc.dram_tensor(k, [1, D], BF16, kind="ExternalInput")

    y_out = nc.dram_tensor("y_out", [SH, D], F32, kind="ExternalOutput")

    # internal DRAM
    cc_k_in_g = [nc.dram_tensor(f"cc_k_in_g{g}", [6 * 128, SH], BF16)
                 for g in range(2)]
    cc_k_out_g = [nc.dram_tensor(f"cc_k_out_g{g}", [N_CORES, 6 * 128, SH], BF16,
                                 addr_space="Shared") for g in range(2)]
    cc_v_in_g = [nc.dram_tensor(f"cc_v_in_g{g}", [4, 128, 4, 128], BF16)
                 for g in range(3)]
    cc_v_out_g = [nc.dram_tensor(f"cc_v_out_g{g}", [N_CORES, 4, 128, 4, 128], BF16,
                                 addr_space="Shared") for g in range(3)]
    scr_rows = nc.dram_tensor("scr_rows", [10, D], F32)
    scr_bf = nc.dram_tensor("scr_bf", [2, D], BF16)
    RG = [list(range(N_CORES))]

    dram = dict(scr_rows=scr_rows, scr_bf=scr_bf, cc_k_in_g=cc_k_in_g, cc_k_out_g=cc_k_out_g,
                cc_v_in_g=cc_v_in_g, cc_v_out_g=cc_v_out_g)

    with tile.TileContext(nc) as tc:
        _body(nc, tc, io, W, C, Rr, y_out, dram, RG)

    nc.compile()
    return nc


def _body(nc, tc, io, W, C, Rr, y_out, dram, RG):
    ctx = {}

    with (
        tc.tile_pool(name="G", bufs=1) as G,
        tc.tile_pool(name="PT", bufs=2, space="PSUM") as PT,   # tr + den
    ):
        # ----- x first, on the (empty) DVE queue so SP can start ctx -----
        x_acc = G.tile([128, 4, D], F32)
        xr = io["x_sh"][:, :].rearrange("(rt p) c -> rt p c", p=128)
        for rt in range(4):
            nc.scalar.dma_start(out=x_acc[:, rt, :], in_=xr[rt])

        # ----- global constants -----
        ident_bf = G.tile([128, 128], BF16)
        make_identity(nc, ident_bf)
        ones_bf = G.tile([128, 1], BF16)
        nc.vector.memset(ones_bf, 1.0)
        ones_f = G.tile([128, 1], F32)
        nc.vector.memset(ones_f, 1.0)
        eps_t = G.tile([128, 1], F32)
        nc.vector.memset(eps_t, EPS)
        kmask_t = G.tile([128, 1], F32)
        nc.scalar.dma_start(out=kmask_t, in_=io["kmask"][:, :])
        cos_b = G.tile([128, SH], BF16)
        nc.scalar.dma_start(out=cos_b, in_=io["cos_dup"][:, :])
        sin_b = G.tile([128, SH], BF16)
        nc.scalar.dma_start(out=sin_b, in_=io["sin_dup"][:, :])
        ctx["cos"], ctx["sin"] = cos_b, sin_b

        BC = {}
        for k, t in C.items():
            BC[k] = G.tile(list(t.shape), F32, tag="bc_" + k, name="bct_" + k)
            nc.scalar.dma_start(out=BC[k], in_=t[:, :])

        def rowbc_ap(a, n=128):
            return bass.AP(tensor=a.tensor, offset=a.offset, ap=[[0, n], [1, D]])

        def colv_ap(a):
            return bass.AP(tensor=a.tensor, offset=a.offset, ap=[[1, 128], [128, H]])

        # shared staging
        hT = G.tile([128, H, SH], BF16)

        def ln_stats(pool, xt):
            stats = pool.tile([128, 3, 6], F32, tag="ln_st", name="ln_st")
            xg = xt.rearrange("p (g f) -> p g f", g=3)
            for g in range(3):
                nc.vector.bn_stats(out=stats[:, g, :], in_=xg[:, g, :])
            mv = pool.tile([128, 2], F32, tag="ln_mv", name="ln_mv")
            nc.vector.bn_aggr(out=mv, in_=stats)
            rstd = pool.tile([128, 1], F32, tag="ln_rs", name="ln_rs")
            nc.scalar.activation(out=rstd, in_=mv[:, 1:2], func=AF.Sqrt, bias=eps_t, scale=1.0)
            nc.vector.reciprocal(out=rstd, in_=rstd)
            return mv, rstd

        def ln_rows(pool, rt):
            """raw LN(x_acc[:,rt,:]) -> rotating row tile (affine in transpose drain).
            The [128,D] apply runs on Act (scale=rstd, bias=-mu*rstd per partition)."""
            xt = x_acc[:, rt, :]
            mv, rstd = ln_stats(pool, xt)
            nb = pool.tile([128, 1], F32, tag="ln_nb", name="ln_nb")
            nc.vector.tensor_scalar(out=nb, in0=mv[:, 0:1], scalar1=rstd,
                                    scalar2=-1.0, op0=ALU.mult, op1=ALU.mult)
            hrow = G.tile([128, D], BF16, tag="hrow", name="hrow", bufs=2)
            nc.scalar.activation(out=hrow, in_=xt, func=AF.Identity, bias=nb, scale=rstd)
            return hrow

        def rows_to_T1(dst_T, hrow, rt, sc_c=None, sh_c=None):
            for dt0 in range(0, H, 4):
                pst = PT.tile([128, 4, 128], BF16, tag="tr", name="pst")
                for j in range(4):
                    nc.tensor.transpose(pst[:, j, :],
                                        hrow[:, (dt0 + j) * 128:(dt0 + j + 1) * 128],
                                        ident_bf)
                if sc_c is None:
                    nc.scalar.copy(
                        out=dst_T[:, dt0:dt0 + 4, rt * 128:(rt + 1) * 128], in_=pst)
                else:
                    for j in range(4):
                        nc.scalar.activation(
                            out=dst_T[:, dt0 + j, rt * 128:(rt + 1) * 128], in_=pst[:, j, :],
                            func=AF.Identity, bias=sh_c[:, dt0 + j:dt0 + j + 1],
                            scale=sc_c[:, dt0 + j:dt0 + j + 1])

        def wtiles(pool, w_dram, mt):
            t = pool.tile([128, H, 128], BF16, tag="wst", name="wst")
            nc.sync.dma_start(out=t, in_=w_dram[mt].rearrange("p (kt c) -> p kt c", c=128))
            return t

        def proj_T(pool, PS, wkey, bkey, dst_f, nfree, rhs_T):
            """dst_f[:, mt, :] = (W.T @ rhs_T) + bias, for 12 output blocks."""
            for mt in range(H):
                wt = wtiles(pool, W[wkey], mt)
                ps = PS.tile([128, nfree], F32, tag="mm", name="ps_p")
                for kt in range(H):
                    nc.tensor.matmul(ps, lhsT=wt[:, kt, :], rhs=rhs_T[:, kt, :],
                                     start=(kt == 0), stop=(kt == H - 1))
                nc.scalar.activation(out=dst_f[:, mt, :], in_=ps, func=AF.Identity,
                                     bias=BC[bkey][:, mt:mt + 1], scale=1.0)

        def proj_rows(pool, PS, wkey, vb_bc, rhs_T, drain, after_dc=None):
            """Row-orientation projection: for each (dc, sb) produce
            ps[128 rows, 512 cols] = rhs_rows @ W[:, dc*512:+512], then
            drain(dc, sb, ps) with bias vb_bc added by the drain."""
            for dc in range(3):
                chains = [PS.tile([128, SH], F32, tag=("mm" if sb < 2 else "mm2"),
                                  name=f"ps_v{dc}{sb}")
                          for sb in range(4)]
                for kt in range(H):
                    wv = pool.tile([128, SH], BF16, tag="wvr", name="wvr", bufs=4)
                    nc.sync.dma_start(out=wv, in_=W[wkey][kt, :, dc * SH:(dc + 1) * SH])
                    for sb in range(4):
                        nc.tensor.matmul(chains[sb], lhsT=rhs_T[:, kt, sb * 128:(sb + 1) * 128],
                                         rhs=wv, start=(kt == 0), stop=(kt == H - 1))
                for sb in range(4):
                    drain(dc, sb, chains[sb], vb_bc)
                if after_dc is not None:
                    after_dc(dc)

        def rms_apply(pool, src_f, nkey, dst_bf, nfree, rope):
            """RMS-normalize src_f (over all 1536 dims, per row) * n-weight,
            optional rope, into dst_bf (allocated here if None, sharing the
            "sq" slot — valid because sq is consumed before dst is written)."""
            sq = pool.tile([128, H, nfree], BF16, tag="sq", name="sq")
            for mt in range(H):
                nc.vector.tensor_tensor(out=sq[:, mt, :], in0=src_f[:, mt, :],
                                        in1=src_f[:, mt, :], op=ALU.mult)
            psd = PT.tile([1, nfree], F32, tag="tr", name="ps_rms")
            for mt in range(H):
                nc.tensor.matmul(psd, lhsT=ones_bf, rhs=sq[:, mt, :],
                                 start=(mt == 0), stop=(mt == H - 1))
            if dst_bf is None:
                dst_bf = pool.tile([128, H, nfree], BF16, tag="sq", name="rms_dst")
            rms = pool.tile([1, nfree], F32, tag="rms", name="rms")
            nc.scalar.activation(out=rms, in_=psd, func=AF.Sqrt, bias=eps_t[0:1, :], scale=1.0 / D)
            rmsb16 = pool.tile([1, nfree], BF16, tag="rms16", name="rms16")
            with nc.allow_low_precision(reason="rms factor ~1, bf16 ok"):
                nc.vector.reciprocal(out=rmsb16, in_=rms)
            rmsb = pool.tile([128, nfree], BF16, tag="rmsb", name="rmsb")
            nc.gpsimd.partition_broadcast(rmsb, rmsb16)
            for mt in range(H):
                if not rope:
                    nc.vector.scalar_tensor_tensor(
                        out=dst_bf[:, mt, :], in0=src_f[:, mt, :],
                        scalar=BC[nkey][:, mt:mt + 1], in1=rmsb, op0=ALU.mult, op1=ALU.mult)
                else:
                    sct = pool.tile([128, nfree], BF16, tag="vrow", name="sct")
                    nc.vector.scalar_tensor_tensor(
                        out=sct, in0=src_f[:, mt, :],
                        scalar=BC[nkey][:, mt:mt + 1], in1=rmsb, op0=ALU.mult, op1=ALU.mult)
                    tec = pool.tile([64, nfree], BF16, tag="rta", name="tec")
                    tos = pool.tile([64, nfree], BF16, tag="rtb", name="tos")
                    nc.vector.tensor_tensor(out=tec, in0=sct[0:64, :], in1=ctx["cos"][0:64, :], op=ALU.mult)
                    nc.vector.tensor_tensor(out=tos, in0=sct[64:128, :], in1=ctx["sin"][64:128, :], op=ALU.mult)
                    nc.vector.tensor_tensor(out=dst_bf[0:64, mt, :], in0=tec, in1=tos, op=ALU.subtract)
                    tes = pool.tile([64, nfree], BF16, tag="rta", name="tes")
                    toc = pool.tile([64, nfree], BF16, tag="rtb", name="toc")
                    nc.vector.tensor_tensor(out=tes, in0=sct[0:64, :], in1=ctx["sin"][0:64, :], op=ALU.mult)
                    nc.vector.tensor_tensor(out=toc, in0=sct[64:128, :], in1=ctx["cos"][64:128, :], op=ALU.mult)
                    nc.vector.tensor_tensor(out=dst_bf[64:128, mt, :], in0=tes, in1=toc, op=ALU.add)
            return dst_bf

        def softmax_av(pool, pool1, PS, q_ap, kt_sl, v_sl, n_kt, masked, dst_ap, acc_bf):
            """One head: scores (paired, so one exp covers two kpos tiles) ->
            AV accumulate; normalized output written to dst_ap [128 hd, 512 q]."""
            adt = BF16 if acc_bf else F32
            ps_a = PS.tile([128, SH], F32, tag="mm", name="ps_at")
            dacc = pool.tile([128, SH], adt, tag="dacc", name="dacc", bufs=2)
            dacc2 = pool.tile([128, SH], adt, tag="dacc2", name="dacc2", bufs=2)
            exps = [None] * n_kt
            npair = n_kt // 2

            def do_st_pair(tp):
                ps2 = PS.tile([128, 2, SH], F32, tag="mm2", name="ps_st2")
                nc.tensor.matmul(ps2[:, 0, :], lhsT=kt_sl(2 * tp), rhs=q_ap,
                                 start=True, stop=True)
                nc.tensor.matmul(ps2[:, 1, :], lhsT=kt_sl(2 * tp + 1), rhs=q_ap,
                                 start=True, stop=True)
                ex2 = pool.tile([128, 2, SH], BF16, tag="exp", name="exp")
                nc.scalar.activation(out=ex2, in_=ps2, func=AF.Exp, bias=0.0, scale=SCALE)
                exps[2 * tp] = ex2[:, 0, :]
                exps[2 * tp + 1] = ex2[:, 1, :]

            def do_st_last():
                ps_s = PS.tile([128, SH], F32, tag="mm", name="ps_st")
                nc.tensor.matmul(ps_s, lhsT=kt_sl(n_kt - 1), rhs=q_ap, start=True, stop=True)
                ex = pool.tile([128, SH], BF16, tag="expl", name="expl", bufs=2)
                nc.scalar.activation(out=ex, in_=ps_s, func=AF.Exp,
                                     bias=kmask_t if masked else 0.0, scale=SCALE)
                exps[n_kt - 1] = ex

            def do_av(t):
                nc.tensor.matmul(ps_a, lhsT=v_sl(t), rhs=exps[t],
                                 start=(t == 0), stop=(t == n_kt - 1))
                if t == 0:
                    nc.vector.tensor_copy(out=dacc, in_=exps[t])
                elif t == 1:
                    nc.vector.tensor_copy(out=dacc2, in_=exps[t])
                elif t % 2 == 0:
                    nc.vector.tensor_add(dacc, dacc, exps[t])
                else:
                    nc.vector.tensor_add(dacc2, dacc2, exps[t])
                exps[t] = None

            for tp in range(npair):
                do_st_pair(tp)
                if tp >= 2:
                    do_av(2 * (tp - 2))
                    do_av(2 * (tp - 2) + 1)
            if n_kt % 2:
                do_st_last()
            for t in range(max(0, 2 * (npair - 2)), n_kt):
                do_av(t)
            if n_kt > 1:
                nc.vector.tensor_add(dacc, dacc, dacc2)
            ps_d = PT.tile([1, SH], F32, tag="tr", name="ps_dn")
            nc.tensor.matmul(ps_d, lhsT=(ones_bf if acc_bf else ones_f), rhs=dacc,
                             start=True, stop=True)
            inv = pool1.tile([1, SH], F32, tag="inv", name="inv", bufs=2)
            nc.vector.reciprocal(out=inv, in_=ps_d)
            invb = pool1.tile([128, SH], F32, tag="invb", name="invb", bufs=2)
            nc.gpsimd.partition_broadcast(invb, inv)
            nc.vector.tensor_tensor(out=dst_ap, in0=ps_a, in1=invb, op=ALU.mult)

        def oproj_residual_w(pool, wpool, PS, wkey, ob_bc, aT_src, gate):
            # o rows: psum[q, cols] = sum_kt aT[:, kt, qsub].T @ Wo[kt][:, cols]
            OC = SH
            for chk in range(D // OC):
                wt = wpool.tile([128, H, OC], BF16, tag="wsto", name="wsto", bufs=2)
                nc.sync.dma_start(
                    out=wt,
                    in_=W[wkey][:, :, chk * OC:(chk + 1) * OC].rearrange("kt p c -> p kt c"))
                sl = slice(chk * OC, (chk + 1) * OC)
                for rt in range(4):
                    ps = PS.tile([128, OC], F32, tag="mm", name="ps_o")
                    for kt in range(H):
                        nc.tensor.matmul(ps, lhsT=aT_src[:, kt, rt * 128:(rt + 1) * 128],
                                         rhs=wt[:, kt, :], start=(kt == 0), stop=(kt == H - 1))
                    u = pool.tile([128, OC], F32, tag="u_o", name="u_o")
                    nc.vector.tensor_tensor(out=u, in0=ps, in1=ob_bc[:, sl], op=ALU.add)
                    if gate is not None:
                        nc.vector.tensor_tensor(out=u, in0=u, in1=gate[:, sl], op=ALU.mult)
                    nc.vector.tensor_tensor(out=x_acc[:, rt, sl], in0=x_acc[:, rt, sl],
                                            in1=u, op=ALU.add)

        # ================= MID scope (sa + ca lifetimes) =================
        with (tc.tile_pool(name="MID", bufs=1) as M,
              tc.tile_pool(name="PM", bufs=1, space="PSUM") as PM_):
            class _PM:
                def tile(self, shape, dtype, tag="mm", name=None):
                    return PM_.tile(shape, dtype, tag=tag, name=name or "psm", bufs=2)
            PM = _PM()

            # --- prologue: processed rows -> DRAM scratch slots ---
            # slots: 0 sc1_msa, 1 sh_msa, 2 g_msa, 3 sh_mlp, 4 sc1_mlp, 5 g_mlp
            def prep_row(scr, tag, row, slot, plus1, bf_slot=None):
                a = scr.tile([1, D], F32, tag="scrA", name="pa_" + tag)
                nc.sync.dma_start(out=a, in_=io["modul"][row:row + 1, :])
                b = scr.tile([1, D], F32, tag="scrB", name="pb_" + tag)
                nc.sync.dma_start(out=b, in_=io["t_mod"][row:row + 1, :])
                nc.vector.tensor_add(a, a, b)
                if plus1:
                    nc.vector.tensor_scalar_add(a, a, 1.0)
                nc.scalar.dma_start(out=dram["scr_rows"][slot:slot + 1, :], in_=a)
                if bf_slot is not None:
                    ab = scr.tile([1, D], BF16, tag="scrC", name="pc_" + tag)
                    nc.vector.tensor_copy(out=ab, in_=a)
                    nc.scalar.dma_start(out=dram["scr_bf"][bf_slot:bf_slot + 1, :], in_=ab)

            def col_tile(pool, tag, dram_row):
                t = pool.tile([128, H], F32, tag="col_" + tag, name="col_" + tag)
                nc.sync.dma_start(out=t, in_=colv_ap(dram_row))
                return t

            def bc_tile(pool, tag, dram_row):
                """broadcast-load a bf16 [1,D] DRAM row into [128,D]."""
                t = pool.tile([128, D], BF16, tag="bc_" + tag, name="bc_" + tag)
                nc.sync.dma_start(out=t, in_=rowbc_ap(dram_row))
                return t

            q_bf = M.tile([128, H, SH], BF16)
            aT = M.tile([128, H, SH], BF16)
            kT_ca = M.tile([128, H, LC], BF16)
            vca = M.tile([128, 4, H, 128], BF16)

            # ---------- Sub1: sa projections + AGs + ca prep ----------
            with tc.tile_pool(name="S1", bufs=1) as S1, tc.tile_pool(name="S1s", bufs=2) as S1s:
                # --- ca context prep: independent of x, fills the startup PE hole ---
                ctx_rows = S1.tile([128, 4, D], BF16, tag="sq", name="ctx_rows")
                nc.sync.dma_start(out=ctx_rows,
                                  in_=io["ctx_bf"][:, :].rearrange("(rt p) c -> p rt c", p=128))
                ctxT = S1.tile([128, H, LC], BF16)
                for rt in range(4):
                    for dt0 in range(0, H, 4):
                        pst = PT.tile([128, 4, 128], BF16, tag="tr", name="pstc")
                        for j in range(4):
                            nc.tensor.transpose(pst[:, j, :],
                                                ctx_rows[:, rt, (dt0 + j) * 128:(dt0 + j + 1) * 128],
                                                ident_bf)
                        nc.scalar.copy(
                            out=ctxT[:, dt0:dt0 + 4, rt * 128:(rt + 1) * 128], in_=pst)

                with tc.tile_pool(name="PRE", bufs=1) as PRE:
                    prep_row(PRE, "sc1_msa", 1, 0, True)
                    prep_row(PRE, "sh_msa", 0, 1, False)
                    sc1_msa_c = col_tile(M, "sc1_msa", dram["scr_rows"][0:1, :])
                    sh_msa_c = col_tile(M, "sh_msa", dram["scr_rows"][1:2, :])

                for rt in range(4):
                    hr = ln_rows(S1, rt)
                    rows_to_T1(hT, hr, rt, sc1_msa_c, sh_msa_c)

                prep_row(S1, "g_msa", 2, 2, False, bf_slot=0)
                prep_row(S1, "sh_mlp", 3, 3, False)
                prep_row(S1, "sc1_mlp", 4, 4, True)
                prep_row(S1, "g_mlp", 5, 5, False, bf_slot=1)

                sh_mlp_c = col_tile(G, "sh_mlp", dram["scr_rows"][3:4, :])
                sc1_mlp_c = col_tile(G, "sc1_mlp", dram["scr_rows"][4:5, :])
                g_mlp = bc_tile(G, "g_mlp", dram["scr_bf"][1:2, :])
                sa_vb_b = bc_tile(S1, "sa_vb", Rr["sa_vb_b16"][:, :])
                ca_vb_b = bc_tile(S1, "ca_vb", Rr["ca_vb_b16"][:, :])
                n3w_c = col_tile(M, "n3w", Rr["n3w_r"][:, :])
                n3b_c = col_tile(M, "n3b", Rr["n3b_r"][:, :])
                b2_b = bc_tile(G, "b2", Rr["b2_b16"][:, :])

                proj_f = S1.tile([128, H, SH], BF16)

                # --- k (rms+rope) + AG ---
                proj_T(S1s, PM, "sa_kw_p", "sa_kb_c", proj_f, SH, hT)
                stage_T = rms_apply(S1, proj_f, "sa_nk_c", None, SH, rope=True)
                # two half AllGathers: heads 0-5 export+gather mid-way through
                # the rope chain, so head-0 scores gate on a half-size gather.
                for g in range(2):
                    nc.scalar.dma_start(
                        out=dram["cc_k_in_g"][g][:, :].rearrange("(mt p) c -> p mt c", p=128),
                        in_=stage_T[:, 6 * g:6 * (g + 1), :])
                    if SIM_MODE:
                        nc.sync.dma_start(out=dram["cc_k_out_g"][g][0],
                                          in_=dram["cc_k_in_g"][g][:, :])
                    else:
                        nc.gpsimd.collective_compute(
                            "AllGather", ALU.bypass, replica_groups=RG,
                            ins=[dram["cc_k_in_g"][g][:, :].opt()],
                            outs=[dram["cc_k_out_g"][g][:, :, :].opt()])

                # --- v (row orientation, straight to DRAM) + AG ---
                def sa_v_drain(dc, sb, ps, vb_bc):
                    vsb = S1.tile([128, SH], BF16, tag="vdr", name="vdr", bufs=2)
                    nc.vector.tensor_tensor(out=vsb, in0=ps, in1=vb_bc[:, dc * SH:(dc + 1) * SH],
                                            op=ALU.add)
                    dst = dram["cc_v_in_g"][dc][0:4, :, sb, :]
                    nc.scalar.dma_start(out=dst.rearrange("h p c -> p h c"),
                                        in_=vsb.rearrange("p (h c) -> p h c", c=128))

                def sa_v_ag(dc):
                    # one 4-head AllGather per projection pass: the first
                    # launches a full pass early, and head h's AV matmuls only
                    # gate on their own quarter-size gather.
                    if SIM_MODE:
                        nc.sync.dma_start(out=dram["cc_v_out_g"][dc][0],
                                          in_=dram["cc_v_in_g"][dc][:, :, :, :])
                    else:
                        nc.gpsimd.collective_compute(
                            "AllGather", ALU.bypass, replica_groups=RG,
                            ins=[dram["cc_v_in_g"][dc][:, :, :, :].opt()],
                            outs=[dram["cc_v_out_g"][dc][:, :, :, :, :].opt()])

                proj_rows(S1s, PM, "sa_vw_r", sa_vb_b, hT, sa_v_drain, after_dc=sa_v_ag)

                # --- q (rms+rope) — overlaps the AGs ---
                proj_T(S1s, PM, "sa_qw_p", "sa_qb_c", proj_f, SH, hT)
                rms_apply(S1, proj_f, "sa_nq_c", q_bf, SH, rope=True)

                # --- ca k/v prep (fills the AG window); both stay in SBUF ---
                proj_T(S1s, PM, "ca_kw", "ca_kb_c", proj_f, LC, ctxT)
                rms_apply(S1, proj_f, "ca_nk_c", kT_ca, LC, rope=False)

                def ca_v_drain(dc, sb, ps, vb_bc):
                    nc.vector.tensor_tensor(
                        out=vca[:, sb, dc * 4:(dc + 1) * 4, :],
                        in0=ps.rearrange("p (h c) -> p h c", c=128),
                        in1=vb_bc[:, dc * SH:(dc + 1) * SH].rearrange("p (h c) -> p h c", c=128),
                        op=ALU.add)

                proj_rows(S1s, PM, "ca_vw_r", ca_vb_b, ctxT, ca_v_drain)

            # ---------- Sub2: self-attention + o-proj + residual ----------
            with tc.tile_pool(name="S2", bufs=1) as S2, tc.tile_pool(name="S2s", bufs=4) as S2s:
                g_msa = bc_tile(S2, "g_msa", dram["scr_bf"][0:1, :])
                sa_ob_b = bc_tile(S2, "sa_ob", Rr["sa_ob_b16"][:, :])

                for h in range(H):
                    k_sb = S2s.tile([128, NKT * 128], BF16, tag="kt", name="k_sb", bufs=2)
                    ko = dram["cc_k_out_g"][h // 6]
                    kh = h % 6
                    nc.sync.dma_start(
                        out=k_sb[:, 0:3584].rearrange("p (co s) -> p co s", s=SH),
                        in_=ko[0:7, kh * 128:(kh + 1) * 128, :].rearrange("co p s -> p co s"))
                    nc.sync.dma_start(out=k_sb[:, 3584:3712],
                                      in_=ko[7, kh * 128:(kh + 1) * 128, 0:128])
                    v_sb = S2s.tile([128, NKT, 128], BF16, tag="vh", name="v_sb", bufs=2)
                    vo = dram["cc_v_out_g"][h // 4]
                    hh = h % 4
                    nc.sync.dma_start(
                        out=v_sb[:, 0:28, :].rearrange("p (co sb) c -> p co sb c", sb=4),
                        in_=vo[0:7, hh, :, :, :].rearrange("co p sb c -> p co sb c"))
                    nc.sync.dma_start(out=v_sb[:, 28, :], in_=vo[7, hh, :, 0, :])
                    softmax_av(S2s, S2, PM, q_bf[:, h, :],
                               lambda t: k_sb[:, t * 128:(t + 1) * 128],
                               lambda t: v_sb[:, t, :],
                               NKT, True, aT[:, h, :], acc_bf=True)
                oproj_residual_w(S2, S2s, PM, "sa_ow", sa_ob_b, aT, g_msa)

            # ---------- Sub3: cross-attention ----------
            with (tc.tile_pool(name="S3", bufs=1) as S3,
                  tc.tile_pool(name="S3s", bufs=4) as S3s,
                  tc.tile_pool(name="S3w", bufs=2) as S3w):
                ca_ob_b = bc_tile(S3, "ca_ob", Rr["ca_ob_b16"][:, :])
                for rt in range(4):
                    hr = ln_rows(S3, rt)
                    rows_to_T1(hT, hr, rt, n3w_c, n3b_c)
                proj_f = S3.tile([128, H, SH], BF16)
                proj_T(S3w, PM, "ca_qw", "ca_qb_c", proj_f, SH, hT)
                rms_apply(S3, proj_f, "ca_nq_c", q_bf, SH, rope=False)

                for h in range(H):
                    softmax_av(S3s, S3, PM, q_bf[:, h, :],
                               lambda t: kT_ca[:, h, t * 128:(t + 1) * 128],
                               lambda t: vca[:, t, h, :],
                               4, False, aT[:, h, :], acc_bf=True)
                oproj_residual_w(S3, S3w, PM, "ca_ow", ca_ob_b, aT, None)

        # ================= FFN (sequence-local) =================
        with (tc.tile_pool(name="FF", bufs=1) as FF,
              tc.tile_pool(name="FFs", bufs=2) as FFs,
              tc.tile_pool(name="PF", bufs=1, space="PSUM") as PF_):
            fdt = FP8 if FFN_FP8 else BF16
            hT8 = FF.tile([128, H, SH], fdt, name="hT8")
            for rt in range(4):
                hr = ln_rows(FF, rt)
                rows_to_T1(hT8, hr, rt, sc1_mlp_c, sh_mlp_c)
            y1 = FF.tile([128, FM, SH], fdt)
            MG = 5 if FFN_FP8 else 2       # m-tiles per w1 load
            for mg in range(FM // MG):
                w1t = FFs.tile([128, MG, H, 128], fdt, tag="w1st", name="w1t", bufs=3)
                nc.scalar.dma_start(out=w1t, in_=W["w1_s"][:, mg * MG:(mg + 1) * MG, :, :])
                for mi in range(MG):
                    m = mg * MG + mi
                    ps = PF_.tile([128, SH], F32, tag="mm", name="ps_f1", bufs=6)
                    if FFN_FP8:
                        for kt2 in range(H // 2):
                            nc.tensor.matmul(
                                ps, lhsT=w1t[:, mi, 2 * kt2:2 * kt2 + 2, :],
                                rhs=hT8[:, 2 * kt2:2 * kt2 + 2, :],
                                start=(kt2 == 0), stop=(kt2 == H // 2 - 1),
                                perf_mode=DROW)
                    else:
                        for kt in range(H):
                            nc.tensor.matmul(ps, lhsT=w1t[:, mi, kt, :], rhs=hT8[:, kt, :],
                                             start=(kt == 0), stop=(kt == H - 1))
                    nc.scalar.activation(out=y1[:, m, :], in_=ps, func=AF.Gelu_apprx_tanh,
                                         bias=BC["b1_c"][:, m:m + 1], scale=1.0)
            # fold g*b2 into the residual once (x += g*b2), so each mm2 drain
            # is just two DVE ops: t = ps*g; y = t + x.
            gb2 = FF.tile([128, D], F32, tag="gb2", name="gb2")
            nc.vector.tensor_tensor(out=gb2, in0=g_mlp, in1=b2_b, op=ALU.mult)
            for sb in range(4):
                nc.vector.tensor_tensor(out=x_acc[:, sb, :], in0=x_acc[:, sb, :],
                                        in1=gb2, op=ALU.add)
            # mm2: k-outer with 4 concurrent PSUM chains (one per s-block),
            # one pass per d-chunk so w2 streams each byte exactly once.
            for dc in range(3):
                sl = slice(dc * SH, (dc + 1) * SH)
                chains = [PF_.tile([128, SH], F32, tag="mm", name=f"ps_f2_{dc}_{sb}", bufs=6)
                          for sb in range(4)]
                KG = 10 if FFN_FP8 else 2      # k-tiles per w2 load
                for kg in range(FM // KG):
                    w2t = FFs.tile([128, KG, SH], fdt, tag="w2st", name="w2t", bufs=3)
                    nc.scalar.dma_start(out=w2t, in_=W["w2_s"][:, kg * KG:(kg + 1) * KG, sl])
                    if FFN_FP8:
                        for kk in range(KG // 2):
                            k2 = kg * KG + 2 * kk
                            for sb in range(4):
                                nc.tensor.matmul(
                                    chains[sb],
                                    lhsT=y1[:, k2:k2 + 2, sb * 128:(sb + 1) * 128],
                                    rhs=w2t[:, 2 * kk:2 * kk + 2, :], start=(k2 == 0),
                                    stop=(k2 == FM - 2), perf_mode=DROW)
                    else:
                        for kk in range(KG):
                            k = kg * KG + kk
                            for sb in range(4):
                                nc.tensor.matmul(chains[sb],
                                                 lhsT=y1[:, k, sb * 128:(sb + 1) * 128],
                                                 rhs=w2t[:, kk, :], start=(k == 0),
                                                 stop=(k == FM - 1))
                for sb in range(4):
                    t1 = FF.tile([128, SH], F32, tag="t1", name="t1", bufs=2)
                    nc.vector.tensor_tensor(out=t1, in0=chains[sb], in1=g_mlp[:, sl], op=ALU.mult)
                    t2 = FF.tile([128, SH], F32, tag="t2", name="t2", bufs=2)
                    nc.vector.tensor_tensor(out=t2, in0=t1, in1=x_acc[:, sb, sl], op=ALU.add)
                    nc.scalar.dma_start(out=y_out[sb * 128:(sb + 1) * 128, sl], in_=t2)


# ---------------- host side ----------------
_NC_CACHE = None


def _get_nc():
    global _NC_CACHE
    if _NC_CACHE is None:
        _NC_CACHE = build()
    return _NC_CACHE


def _prep(inputs):
    f32 = np.float32
    perm_head = np.concatenate([np.arange(0, 128, 2), np.arange(1, 128, 2)])
    full_perm = np.concatenate([128 * h + perm_head for h in range(H)])

    x = np.asarray(inputs["x"], f32).reshape(S, D)
    x_pad = np.zeros((SP, D), f32)
    x_pad[:S] = x
    ctx_b = np.asarray(inputs["context"], f32).reshape(LC, D).astype(BF)
    modul = np.asarray(inputs["modulation"], f32).reshape(6, D)
    t_mod = np.asarray(inputs["t_mod"], f32).reshape(6, D)

    cos = np.asarray(inputs["rope_cos"], f32)
    sin = np.asarray(inputs["rope_sin"], f32)
    cos_pad = np.ones((SP, 64), f32)
    sin_pad = np.zeros((SP, 64), f32)
    cos_pad[:S] = cos
    sin_pad[:S] = sin

    kmask = np.where(np.arange(128) < 16, 0.0, NEG).astype(f32).reshape(128, 1)

    def colmat(v, perm=None):
        v = np.asarray(v, f32).reshape(D)
        if perm is not None:
            v = v[perm]
        return np.ascontiguousarray(v.reshape(H, 128).T)

    def wtile(w):
        # [1536,1536] -> [mt, p, kt, c] with tile[mt, p, kt*128+c] = W[kt*128+p, mt*128+c]
        w = np.asarray(w, f32).reshape(H, 128, H, 128)
        return np.ascontiguousarray(w.transpose(2, 1, 0, 3).reshape(H, 128, D)).astype(BF)

    def wrow(w):
        # [1536,1536] -> [kt, p, d] with tile[kt, p, d] = W[kt*128+p, d]
        return np.ascontiguousarray(np.asarray(w, f32).reshape(H, 128, D)).astype(BF)

    w1 = np.asarray(inputs["ffn_w1"], f32)
    w2 = np.asarray(inputs["ffn_w2"], f32)
    b1 = np.asarray(inputs["ffn_b1"], f32)

    shared = dict(
        ctx_bf=ctx_b, modul=modul, t_mod=t_mod, kmask=kmask,
        sa_qw_p=wtile(np.asarray(inputs["sa_qw"], f32)[:, full_perm]),
        sa_kw_p=wtile(np.asarray(inputs["sa_kw"], f32)[:, full_perm]),
        sa_vw_r=wrow(inputs["sa_vw"]),
        sa_ow=np.asarray(inputs["sa_ow"], f32).reshape(H, 128, D).astype(BF),
        ca_qw=wtile(inputs["ca_qw"]),
        ca_kw=wtile(inputs["ca_kw"]),
        ca_vw_r=wrow(inputs["ca_vw"]),
        ca_ow=np.asarray(inputs["ca_ow"], f32).reshape(H, 128, D).astype(BF),
        sa_qb_c=colmat(inputs["sa_qb"], full_perm),
        sa_kb_c=colmat(inputs["sa_kb"], full_perm),
        sa_nq_c=colmat(inputs["sa_nq"], full_perm),
        sa_nk_c=colmat(inputs["sa_nk"], full_perm),
        ca_qb_c=colmat(inputs["ca_qb"]),
        ca_kb_c=colmat(inputs["ca_kb"]),
        ca_nq_c=colmat(inputs["ca_nq"]),
        ca_nk_c=colmat(inputs["ca_nk"]),
        sa_ob_b16=np.asarray(inputs["sa_ob"], f32).reshape(1, D).astype(BF),
        ca_ob_b16=np.asarray(inputs["ca_ob"], f32).reshape(1, D).astype(BF),
        sa_vb_b16=np.asarray(inputs["sa_vb"], f32).reshape(1, D).astype(BF),
        ca_vb_b16=np.asarray(inputs["ca_vb"], f32).reshape(1, D).astype(BF),
        n3w_r=np.asarray(inputs["n3_w"], f32).reshape(1, D),
        n3b_r=np.asarray(inputs["n3_b"], f32).reshape(1, D),
        b2_b16=np.asarray(inputs["ffn_b2"], f32).reshape(1, D).astype(BF),
        w1_s=np.ascontiguousarray(
            w1.reshape(H, 128, FM, 128).transpose(1, 2, 0, 3)).astype(F8 if FFN_FP8 else BF),
        w2_s=np.ascontiguousarray(
            w2.reshape(FM, 128, D).transpose(1, 0, 2)).astype(F8 if FFN_FP8 else BF),
        b1_c=np.ascontiguousarray(b1.reshape(FM, 128).T),
    )

    in_maps = []
    for c in range(N_CORES):
        ct = cos_pad[c * SH:(c + 1) * SH].T
        st = sin_pad[c * SH:(c + 1) * SH].T
        m = dict(shared)
        m.update(
            x_sh=np.ascontiguousarray(x_pad[c * SH:(c + 1) * SH]),
            cos_dup=np.ascontiguousarray(np.concatenate([ct, ct], axis=0)).astype(BF),
            sin_dup=np.ascontiguousarray(np.concatenate([st, st], axis=0)).astype(BF),
        )
        in_maps.append(m)
    return in_maps


def kernel(**inputs):
    nc = _get_nc()
    in_maps = _prep(inputs)
    res = run_bass_kernel_spmd(nc, in_maps, core_ids=list(range(N_CORES)))
    out = np.concatenate([res.results[c]["y_out"] for c in range(N_CORES)], axis=0)[:S]
    return out.reshape(1, S, D).astype(np.float32)

